# revision 100
# baseline (speedup 1.0000x reference)
"""Trainium2 Bass kernel for point-cloud ball-query attention.

Shapes (hardcoded): b=2, l=4, n=1024, dim=512, heads=8, dim_head=64,
radius=0.2, nsample=8.  Sharded over 8 NeuronCores: core c handles
(batch b = c // 4, query frame i = c % 4) and produces out[b, i].

Structure: frame-major software pipeline.  For each key frame f:
LN+KV projection (PE) -> kv rows to DRAM, ball query (ACT squares +
DVE/Pool adds + DVE top-8) -> frame-local neighbor indices -> wrapped
int16 gather tables -> 2304 B row gathers -> per-(query-tile, frame)
attention partials (unnormalized e = exp(logit), e*v accumulated across
frames).  A final per-query-tile pass normalizes, adds the spatial
branch, and the epilogue does out-projection + gelu + residual with all
gelus batched at the end (one ACT table switch).

Layouts:
- K rows (h,d)-major, V rows (d,h)-major (host permutes wkv columns),
  xyz fp16 appended: kv row = [k 512 | v 512 | xyz 3 | pad] = 1152 fp16
  (2304 B, a multiple of the gather's 256 B granule).
- Hot elementwise ops are fp16 tensor_tensor with packed innermost
  operands (DVE 2x mode); broadcasts sit on non-innermost axes.
"""

import numpy as np

B, L, N, DIM = 2, 4, 1024, 512
H, DH = 8, 64
INNER = H * DH
NS = 8
LNS = L * NS  # 32 neighbors per query
R2 = float(np.float32(0.2) ** 2)
EPS = 1e-5
QT = N // 128  # 8 query tiles per core
ROW = 1152  # fp16 elems per kv row (2304 B)

_CACHE = {}


def _build_program(debug=False):
    import contextlib

    import concourse.bass as bass
    import concourse.tile as tile
    from concourse import bacc, mybir

    f32 = mybir.dt.float32
    f16 = mybir.dt.float16
    i16 = mybir.dt.int16
    AF = mybir.ActivationFunctionType
    OP = mybir.AluOpType

    nc = bacc.Bacc(None, target_bir_lowering=False)

    # ---- I/O ----
    xyzq = nc.dram_tensor("xyzq", [N, 3], f32, kind="ExternalInput")
    xyz_all = nc.dram_tensor("xyz_all", [L * N, 3], f32, kind="ExternalInput")
    xyz16 = nc.dram_tensor("xyz16", [L * N, 3], f16, kind="ExternalInput")
    feat16 = nc.dram_tensor("feat16", [L * N, DIM], f16, kind="ExternalInput")
    featq16 = nc.dram_tensor("featq16", [N, DIM], f16, kind="ExternalInput")
    wq = nc.dram_tensor("wq", [128, 4, INNER], f16, kind="ExternalInput")
    wkv = nc.dram_tensor("wkv", [128, 4, 2 * INNER], f16, kind="ExternalInput")
    wout = nc.dram_tensor("wout", [128, 4, DIM], f16, kind="ExternalInput")
    wspe_d = nc.dram_tensor("wspe", [1, 3 * DH * H], f16, kind="ExternalInput")
    bout = nc.dram_tensor("bout", [1, DIM], f16, kind="ExternalInput")
    desc_d = nc.dram_tensor("desc", [1, N], f32, kind="ExternalInput")
    out_frame = nc.dram_tensor("out_frame", [N, DIM], f16, kind="ExternalOutput")
    if debug:
        dbg_idx = nc.dram_tensor("dbg_idx", [N, LNS], f32, kind="ExternalOutput")

    kvf = [nc.dram_tensor(f"kv{f}", [N, ROW], f16) for f in range(L)]
    idxb = nc.dram_tensor("idxb", [L, QT, NS * 128], i16)

    with tile.TileContext(nc) as tc:
        ctx = contextlib.ExitStack()
        with ctx:
            singles = ctx.enter_context(tc.tile_pool(name="singles", bufs=1))

            # ---- weights / constants ----
            wq_sb = singles.tile([128, 4, INNER], f16)
            nc.sync.dma_start(out=wq_sb[:], in_=wq[:])
            wkv_sb = singles.tile([128, 4, 2 * INNER], f16)
            nc.scalar.dma_start(out=wkv_sb[:], in_=wkv[:])
            wout_sb = singles.tile([128, 4, DIM], f16)
            nc.sync.dma_start(out=wout_sb[:], in_=wout[:])
            wspe = singles.tile([128, 3, DH, H], f16)
            nc.sync.dma_start(
                out=wspe[:], in_=bass.AP(wspe_d, 0, [[0, 128], [1, 3 * DH * H]])
            )
            boutb = singles.tile([1, DIM], f16)
            nc.sync.dma_start(out=boutb[:], in_=bout[:])
            ones1 = singles.tile([1, 128], f16)
            nc.vector.memset(ones1[:], 1.0)
            epsb = singles.tile([128, 1], f32)
            nc.vector.memset(epsb[:], EPS)
            descb = singles.tile([128, N], f32)
            nc.sync.dma_start(out=descb[0:1, :], in_=desc_d[:])
            nc.gpsimd.partition_broadcast(descb[:], descb[0:1, :])

            # persistent per-core activations
            q16 = singles.tile([128, QT, INNER], f16)
            featq = singles.tile([128, QT, DIM], f16)
            qx16 = singles.tile([128, QT, 3], f32)
            nc.scalar.dma_start(
                out=qx16[:], in_=bass.AP(xyzq, 0, [[3, 128], [3 * 128, QT], [1, 3]])
            )
            # all frames' xyz fp16 staged on-chip (row t*128+p -> [p, t, :])
            xyzsb = singles.tile([128, L * QT, 3], f16)
            nc.scalar.dma_start(
                out=xyzsb[:],
                in_=bass.AP(xyz16, 0, [[3, 128], [3 * 128, L * QT], [1, 3]]),
            )
            nc.sync.dma_start(
                out=featq[:],
                in_=bass.AP(featq16, 0, [[DIM, 128], [DIM * 128, QT], [1, DIM]]),
            )

            # per-query-tile attention state, alive across the frame stream
            e_all = singles.tile([128, QT, LNS, H], f16)
            eT = singles.tile([128, QT, H, LNS], f16)
            dispT = singles.tile([128, QT, 3, LNS], f16)
            accev = singles.tile([128, QT, DH, H], f16)
            fr_all = singles.tile([128, QT, INNER], f16)

            # ---- pools ----
            ln = ctx.enter_context(tc.tile_pool(name="ln", bufs=3))
            lnst = ctx.enter_context(tc.tile_pool(name="lnst", bufs=2))
            xp = ctx.enter_context(tc.tile_pool(name="xp", bufs=2))
            tpsum = ctx.enter_context(tc.tile_pool(name="tpsum", bufs=2, space="PSUM"))
            qpsum = ctx.enter_context(tc.tile_pool(name="qpsum", bufs=1, space="PSUM"))
            kvo = ctx.enter_context(tc.tile_pool(name="kvo", bufs=2))
            refp = ctx.enter_context(tc.tile_pool(name="refp", bufs=2))
            bqs = ctx.enter_context(tc.tile_pool(name="bqs", bufs=1))
            bqa = ctx.enter_context(tc.tile_pool(name="bqa", bufs=2))
            bqd = ctx.enter_context(tc.tile_pool(name="bqd", bufs=2))
            ttp = ctx.enter_context(tc.tile_pool(name="ttp", bufs=2))
            gat = ctx.enter_context(tc.tile_pool(name="gat", bufs=2))
            att = ctx.enter_context(tc.tile_pool(name="att", bufs=3))
            fin = ctx.enter_context(tc.tile_pool(name="fin", bufs=2))

            def frame_block(f):
                """LN + projections for one frame (f == -1: query frame)."""
                mvall = lnst.tile([128, QT, 2], f32, tag="mv")
                if f < 0:
                    xf = featq
                else:
                    xf = xp.tile([128, QT, DIM], f16, tag="x", name="x")
                    eng = nc.sync if f % 2 == 0 else nc.scalar
                    eng.dma_start(
                        out=xf[:],
                        in_=bass.AP(
                            feat16, f * N * DIM,
                            [[DIM, 128], [DIM * 128, QT], [1, DIM]],
                        ),
                    )
                for t in range(QT):
                    stats = lnst.tile([128, 6], f32, tag="st")
                    nc.vector.bn_stats(out=stats[:], in_=xf[:, t, :])
                    nc.vector.bn_aggr(out=mvall[:, t, :], in_=stats[:])
                rstd8 = lnst.tile([128, QT], f32, tag="rstd")
                nc.scalar.activation(
                    out=rstd8[:], in_=mvall[:, :, 1], func=AF.Sqrt,
                    bias=epsb[:], scale=1.0,
                )
                nc.vector.reciprocal(out=rstd8[:], in_=rstd8[:])
                for t in range(QT):
                    xn = ln.tile([128, DIM], f16, tag="xn")
                    nc.vector.tensor_scalar(
                        out=xn[:], in0=xf[:, t, :], scalar1=mvall[:, t, 0:1],
                        scalar2=rstd8[:, t : t + 1], op0=OP.subtract, op1=OP.mult,
                    )
                    xnT = ln.tile([128, 4, 128], f16, tag="xnT")
                    with tc.high_priority():
                        nc.sync.dma_start_transpose(xnT[:], xn[:])
                    if f < 0:
                        ps = qpsum.tile([128, INNER], f32, tag="qps")
                        for c in range(4):
                            nc.tensor.matmul(
                                out=ps[:], lhsT=xnT[:, c, :], rhs=wq_sb[:, c, :],
                                start=(c == 0), stop=(c == 3),
                            )
                        nc.scalar.activation(out=q16[:, t, :], in_=ps[:], func=AF.Copy)
                    else:
                        kv16 = kvo.tile([128, 2 * INNER + 3], f16, tag="kv16")
                        ps = tpsum.tile([128, 2 * INNER], f32, tag="kvps")
                        for half in range(2):
                            sl = slice(half * INNER, (half + 1) * INNER)
                            for c in range(4):
                                nc.tensor.matmul(
                                    out=ps[:, sl], lhsT=xnT[:, c, :],
                                    rhs=wkv_sb[:, c, sl],
                                    start=(c == 0), stop=(c == 3),
                                )
                        with tc.high_priority(offset=2 * 10**6):
                            if f == 0:
                                nc.vector.tensor_copy(
                                    out=kv16[:, 0 : 2 * INNER], in_=ps[:]
                                )
                            else:
                                nc.scalar.activation(
                                    out=kv16[:, 0 : 2 * INNER], in_=ps[:], func=AF.Copy
                                )
                        nc.vector.tensor_copy(
                            out=kv16[:, 2 * INNER :], in_=xyzsb[:, f * QT + t, :]
                        )
                        eng2 = nc.scalar if t % 2 == 0 else nc.sync
                        with tc.high_priority():
                            eng2.dma_start(
                                out=bass.AP(
                                    kvf[f], (t * 128) * ROW,
                                    [[ROW, 128], [1, 2 * INNER + 3]],
                                ),
                                in_=kv16[:],
                            )

            def bq_frame(f, refb):
                """Ball query of all query tiles against frame f; builds the
                frame's wrapped int16 gather table."""
                vals = bqd.tile([128, QT, NS], f32, tag="vals")
                for qt in range(QT):
                    qneg = bqd.tile([128, 3], f32, tag="qneg")
                    nc.vector.tensor_scalar(
                        out=qneg[:], in0=qx16[:, qt, :], scalar1=-1.0, scalar2=None,
                        op0=OP.mult,
                    )
                    acc = bqa.tile([128, N], f32, tag="acc")
                    sqb = bqs.tile([128, N], f32, tag="sqb")
                    sqc = bqs.tile([128, N], f32, tag="sqc")
                    nc.scalar.activation(
                        out=acc[:], in_=refb[:, 0, :], func=AF.Square,
                        bias=qneg[:, 0:1], scale=1.0,
                    )
                    nc.scalar.activation(
                        out=sqb[:], in_=refb[:, 1, :], func=AF.Square,
                        bias=qneg[:, 1:2], scale=1.0,
                    )
                    nc.scalar.activation(
                        out=sqc[:], in_=refb[:, 2, :], func=AF.Square,
                        bias=qneg[:, 2:3], scale=1.0,
                    )
                    # distance adds on Pool for most tiles (it is otherwise
                    # idle during the query phase); the fused in-radius score
                    # select must run on DVE (TensorScalarPtr is DVE-only)
                    eng = nc.gpsimd
                    eng.tensor_tensor(out=acc[:], in0=acc[:], in1=sqb[:], op=OP.add)
                    eng.tensor_tensor(out=acc[:], in0=acc[:], in1=sqc[:], op=OP.add)
                    # score = (d2 < R2) * (N - j): top-8 = 8 lowest in-radius j
                    nc.vector.scalar_tensor_tensor(
                        out=acc[:], in0=acc[:], scalar=R2, in1=descb[:],
                        op0=OP.is_lt, op1=OP.mult,
                    )
                    with tc.high_priority(offset=5 * 10**6):
                        nc.vector.max(out=vals[:, qt, :], in_=acc[:])
                # decode to frame-local indices (two halves so the first
                # gather tables are available before all top-8s finish)
                ttf = ttp.tile([128, QT, 64], i16, tag="tt", name="ttf")
                HQ = QT // 2
                for h0 in range(2):
                    qsl = slice(h0 * HQ, (h0 + 1) * HQ)
                    valid = bqd.tile([128, HQ, NS], f32, tag="valid")
                    nc.vector.tensor_scalar(
                        out=valid[:], in0=vals[:, qsl], scalar1=0.0, scalar2=None,
                        op0=OP.is_gt,
                    )
                    jdx = bqd.tile([128, HQ, NS], f32, tag="jdx")
                    nc.vector.tensor_scalar(
                        out=jdx[:], in0=vals[:, qsl], scalar1=-1.0, scalar2=float(N),
                        op0=OP.mult, op1=OP.add,
                    )
                    first = bqd.tile([128, HQ, 1], f32, tag="first")
                    nc.vector.tensor_tensor(
                        out=first[:], in0=jdx[:, :, 0:1], in1=valid[:, :, 0:1],
                        op=OP.mult,
                    )
                    pad = bqd.tile([128, HQ, NS], f32, tag="pad")
                    nc.vector.tensor_tensor(
                        out=pad[:], in0=jdx[:],
                        in1=first[:].broadcast_to([128, HQ, NS]), op=OP.subtract,
                    )
                    nc.vector.tensor_tensor(
                        out=pad[:], in0=pad[:], in1=valid[:], op=OP.mult
                    )
                    nc.vector.tensor_tensor(
                        out=pad[:], in0=pad[:],
                        in1=first[:].broadcast_to([128, HQ, NS]), op=OP.add,
                    )
                    idx16 = bqd.tile([128, HQ * NS], i16, tag="idx16")
                    nc.vector.tensor_scalar(
                        out=idx16[:], in0=pad[:].rearrange("p q s -> p (q s)"),
                        scalar1=0.0, scalar2=None, op0=OP.add,
                    )
                    if debug:
                        padg = bqd.tile([128, HQ, NS], f32, tag="padg")
                        nc.vector.tensor_scalar(
                            out=padg[:], in0=pad[:], scalar1=float(f * N),
                            scalar2=None, op0=OP.add,
                        )
                        for qh in range(HQ):
                            qt = h0 * HQ + qh
                            nc.sync.dma_start(
                                out=bass.AP(
                                    dbg_idx, qt * 128 * LNS + f * NS,
                                    [[LNS, 128], [1, NS]],
                                ),
                                in_=padg[:, qh, :],
                            )
                    # wrapped gather table: per (f, qt) compact block,
                    # dram[f*8192+qt*1024+p*64+s*8+g] = idx16[q=16g+p, qt*8+s]
                    with tc.high_priority():
                        for qh in range(HQ):
                            qt = h0 * HQ + qh
                            off = f * (QT * NS * 128) + qt * (NS * 128)
                            nc.sync.dma_start(
                                out=bass.AP(idxb, off, [[1, 8], [64, 16], [8, NS]]),
                                in_=idx16[:, qh * NS : (qh + 1) * NS],
                            )
                            nc.scalar.dma_start(
                                out=ttf[:, qt, :],
                                in_=bass.AP(idxb, off, [[0, 8], [64, 16], [1, 64]]),
                            )
                return ttf

            def attn_frame(f, qt, ttf):
                """Gather + attention partials for (query tile qt, frame f)."""
                kvg = gat.tile([128, NS, ROW], f16, tag="kvg")
                with tc.high_priority(offset=10**6):
                    nc.gpsimd.dma_gather(
                        out_ap=kvg[:], in_ap=kvf[f][:], idxs_ap=ttf[:, qt, :],
                        num_idxs=NS * 128, num_idxs_reg=NS * 128, elem_size=ROW,
                    )
                kview = kvg[:, :, 0:INNER].rearrange("p s (h d) -> p s h d", d=DH)
                # logits: prod = k * q, tree-reduce over d
                prod = att.tile([128, NS, H, DH], f16, tag="work")
                qb = (
                    q16[:, qt, :]
                    .rearrange("p (h d) -> p h d", d=DH)
                    .unsqueeze(1)
                    .broadcast_to([128, NS, H, DH])
                )
                nc.vector.tensor_tensor(out=prod[:], in0=kview, in1=qb, op=OP.mult)
                w = DH
                while w > 1:
                    w //= 2
                    nc.vector.tensor_tensor(
                        out=prod[:, :, :, 0:w], in0=prod[:, :, :, 0:w],
                        in1=prod[:, :, :, w : 2 * w], op=OP.add,
                    )
                logits = prod[:, :, :, 0]
                with tc.high_priority(offset=10**7):
                    nc.scalar.activation(
                        out=e_all[:, qt, f * NS : (f + 1) * NS, :], in_=logits,
                        func=AF.Exp,
                    )
                nc.scalar.activation(
                    out=eT[:, qt, :, f * NS : (f + 1) * NS],
                    in_=logits.transpose([0, 2, 1]), func=AF.Exp,
                )
                # unnormalized e*v accumulation (v is (d,h)-major)
                vview = kvg[:, :, INNER : 2 * INNER].rearrange(
                    "p s (d h) -> p s d h", h=H
                )
                ev = att.tile([128, NS, DH, H], f16, tag="work")
                eb = (
                    e_all[:, qt, f * NS : (f + 1) * NS, :]
                    .unsqueeze(2)
                    .broadcast_to([128, NS, DH, H])
                )
                nc.vector.tensor_tensor(out=ev[:], in0=vview, in1=eb, op=OP.mult)
                w = NS
                while w > 1:
                    w //= 2
                    nc.vector.tensor_tensor(
                        out=ev[:, 0:w], in0=ev[:, 0:w], in1=ev[:, w : 2 * w], op=OP.add
                    )
                if f == 0:
                    nc.vector.tensor_copy(out=accev[:, qt], in_=ev[:, 0])
                else:
                    nc.vector.tensor_tensor(
                        out=accev[:, qt], in0=accev[:, qt], in1=ev[:, 0], op=OP.add
                    )
                # displacement (gathered xyz - query xyz), coord-major
                for c in range(3):
                    nc.vector.tensor_scalar(
                        out=dispT[:, qt, c, f * NS : (f + 1) * NS],
                        in0=kvg[:, :, 2 * INNER + c],
                        scalar1=qx16[:, qt, c : c + 1], scalar2=None,
                        op0=OP.subtract,
                    )

            def finalize(qt):
                """Softmax normalization + spatial branch -> fr_all[qt]."""
                zs = fin.tile([128, 16, H], f16, tag="zs")
                ef = e_all[:, qt]
                nc.gpsimd.tensor_tensor(
                    out=zs[:], in0=ef[:, 0:16, :], in1=ef[:, 16:32, :], op=OP.add
                )
                w = 16
                while w > 1:
                    w //= 2
                    nc.gpsimd.tensor_tensor(
                        out=zs[:, 0:w, :], in0=zs[:, 0:w, :],
                        in1=zs[:, w : 2 * w, :], op=OP.add,
                    )
                rz = fin.tile([128, H], f16, tag="rz")
                nc.vector.reciprocal(out=rz[:], in_=zs[:, 0, :])

                # spatial: m[c,h] = max_j e*disp; dproj = sum_c wspe[c] * m[c]*rz
                p3 = fin.tile([128, 3, H, LNS], f16, tag="p3")
                nc.vector.tensor_tensor(
                    out=p3[:],
                    in0=dispT[:, qt].unsqueeze(2).broadcast_to([128, 3, H, LNS]),
                    in1=eT[:, qt].unsqueeze(1).broadcast_to([128, 3, H, LNS]),
                    op=OP.mult,
                )
                w = LNS
                while w > 1:
                    w //= 2
                    nc.vector.tensor_tensor(
                        out=p3[:, :, :, 0:w], in0=p3[:, :, :, 0:w],
                        in1=p3[:, :, :, w : 2 * w], op=OP.max,
                    )
                mr = fin.tile([128, 3, H], f16, tag="mr")
                nc.vector.tensor_tensor(
                    out=mr[:], in0=p3[:, :, :, 0],
                    in1=rz[:].unsqueeze(1).broadcast_to([128, 3, H]), op=OP.mult,
                )
                dp = fin.tile([128, DH, H], f16, tag="dp")
                nc.vector.tensor_tensor(
                    out=dp[:], in0=wspe[:, 0],
                    in1=mr[:, 0, :].unsqueeze(1).broadcast_to([128, DH, H]),
                    op=OP.mult,
                )
                for c in (1, 2):
                    t2 = fin.tile([128, DH, H], f16, tag="sp2")
                    nc.vector.tensor_tensor(
                        out=t2[:], in0=wspe[:, c],
                        in1=mr[:, c, :].unsqueeze(1).broadcast_to([128, DH, H]),
                        op=OP.mult,
                    )
                    nc.vector.tensor_tensor(out=dp[:], in0=dp[:], in1=t2[:], op=OP.add)
                # fr = accev * rz + dproj   (all (d,h)-major)
                frv = fr_all[:, qt, :].rearrange("p (d h) -> p d h", h=H)
                nc.vector.tensor_tensor(
                    out=frv, in0=accev[:, qt],
                    in1=rz[:].unsqueeze(1).broadcast_to([128, DH, H]), op=OP.mult,
                )
                nc.vector.tensor_tensor(out=frv, in0=frv, in1=dp[:], op=OP.add)

            # ============ emission: frame-major pipeline ============
            def prep_frame(f):
                frame_block(f)
                refb = refp.tile([128, 3 * N], f32, tag="ref", name="refb")
                nc.sync.dma_start(
                    out=refb[0:1, :],
                    in_=bass.AP(xyz_all, f * 3 * N, [[0, 1], [1, 3 * N]]),
                )
                nc.gpsimd.partition_broadcast(refb[:], refb[0:1, :])
                refv = bass.AP(refb.tensor, refb.offset, [refb.ap[0], [1, 3], [3, N]])
                return bq_frame(f, refv)

            # ball query + projections run one frame ahead of attention
            with nc.allow_low_precision("fp16 attention pipeline"):
                frame_block(-1)
                tts = [prep_frame(0)]
                for f in range(L):
                    if f + 1 < L:
                        tts.append(prep_frame(f + 1))
                    for qt in range(QT):
                        attn_frame(f, qt, tts[f])
                for qt in range(QT):
                    finalize(qt)

                # ---- out projection + gelu + residual (gelus batched) ----
                epi = ctx.enter_context(tc.tile_pool(name="epi", bufs=2))
                episum = ctx.enter_context(
                    tc.tile_pool(name="episum", bufs=2, space="PSUM")
                )
                for qt in range(QT):
                    frT = epi.tile([128, 4, 128], f16, tag="frT")
                    nc.sync.dma_start_transpose(frT[:], fr_all[:, qt, :])
                    ps = episum.tile([128, DIM], f32, tag="ops")
                    for c in range(4):
                        nc.tensor.matmul(
                            out=ps[:], lhsT=frT[:, c, :], rhs=wout_sb[:, c, :],
                            start=(c == 0), stop=False,
                        )
                    nc.tensor.matmul(
                        out=ps[:], lhsT=ones1[:], rhs=boutb[:], start=False, stop=True
                    )
                    g16 = epi.tile([128, DIM], f16, tag="g16")
                    nc.scalar.activation(out=g16[:], in_=ps[:], func=AF.Gelu)
                    nc.vector.tensor_tensor(
                        out=g16[:], in0=g16[:], in1=featq[:, qt, :], op=OP.add
                    )
                    eng = nc.sync if qt % 2 == 0 else nc.scalar
                    eng.dma_start(
                        out=out_frame[qt * 128 : (qt + 1) * 128, :], in_=g16[:]
                    )

    nc.finalize()
    return nc


def _prep_inputs(inputs, core):
    xyzs = np.asarray(inputs["xyzs"], np.float32)
    feature = np.asarray(inputs["feature"], np.float32)
    gamma = np.asarray(inputs["gamma"], np.float32)
    beta = np.asarray(inputs["beta"], np.float32)
    w_qkv = np.asarray(inputs["w_qkv"], np.float32)
    w_spatial = np.asarray(inputs["w_spatial"], np.float32)
    w_out = np.asarray(inputs["w_out"], np.float32)
    b_out = np.asarray(inputs["b_out"], np.float32)
    assert not np.any(beta), "kernel assumes beta == 0 (as in setup_inputs)"

    b, i = core // L, core % L
    scale = DH ** -0.5
    wg = gamma[:, None] * w_qkv
    wq_ = (wg[:, :INNER] * scale).astype(np.float16)
    wk = wg[:, INNER : 2 * INNER].astype(np.float16)
    wv = wg[:, 2 * INNER :].astype(np.float16)
    # v columns permuted to (d, h)-major
    wv = wv.reshape(DIM, H, DH).transpose(0, 2, 1).reshape(DIM, INNER)
    wkv_ = np.concatenate([wk, wv], axis=1)
    # w_out rows permuted to (d, h)-major
    wo = w_out.reshape(H, DH, DIM).transpose(1, 0, 2).reshape(INNER, DIM)

    def chunk(w):  # (512, O) -> (128, 4, O) with dims d = 128*c + p
        return np.ascontiguousarray(
            w.reshape(4, 128, w.shape[1]).transpose(1, 0, 2)
        ).astype(np.float16)

    wspe = (
        np.broadcast_to(w_spatial.reshape(3, DH, 1), (3, DH, H))
        .reshape(1, 3 * DH * H)
        .astype(np.float16)
    )

    xyz_b = np.ascontiguousarray(xyzs[b].reshape(L * N, 3))
    feat_b = np.ascontiguousarray(feature[b].reshape(L * N, DIM)).astype(np.float16)
    return {
        "xyzq": np.ascontiguousarray(xyzs[b, i]),
        "xyz_all": xyz_b,
        "xyz16": xyz_b.astype(np.float16),
        "feat16": feat_b,
        "featq16": np.ascontiguousarray(feat_b[i * N : (i + 1) * N]),
        "wq": chunk(wq_),
        "wkv": chunk(wkv_),
        "wout": chunk(wo),
        "wspe": wspe,
        "bout": b_out.reshape(1, DIM).astype(np.float16),
        "desc": (float(N) - np.arange(N, dtype=np.float32)).reshape(1, N),
    }


def kernel(**inputs):
    from concourse.bass_utils import run_bass_kernel_spmd

    debug = bool(inputs.pop("_debug", False))
    key = ("prog", debug)
    if key not in _CACHE:
        _CACHE[key] = _build_program(debug=debug)
    nc = _CACHE[key]

    in_maps = [_prep_inputs(inputs, c) for c in range(B * L)]
    res = run_bass_kernel_spmd(nc, in_maps, list(range(B * L)), trace=False)
    out = np.stack(
        [np.asarray(res.results[c]["out_frame"], np.float32) for c in range(B * L)],
        axis=0,
    ).reshape(B, L, N, DIM)
    if debug:
        kernel._dbg = [np.asarray(res.results[c]["dbg_idx"]) for c in range(B * L)]
    return out.astype(np.float32)
